# revision 5
# baseline (speedup 1.0000x reference)
"""Self-contained Trainium2 Bass kernel for nn_CA_9363028705415 (sparse_attention).

Computes, per batch b:
    Q = relu(x[b] @ qW1 + qb1) @ qW2 + qb2          # [M, K]
    Kt = relu(x[b] @ kW1 + kb1) @ kW2 + kb2         # [M, K]
    S = Q @ Kt.T                                    # [M, M]
    out[b] = softmax(S / rowmax(S), axis=-1)        # max-DIVISION normalization

Shapes: B=16, M=2048, D=128, H=256, K=64.  Output [16, 2048, 2048] f32 (256 MB).

Sharding: data-parallel over batch across 8 NeuronCores; 2 batches/core; tiny
MLP weights replicated.  Single NEFF run SPMD via run_bass_kernel_spmd.

Device writes the output in fp16 (16 MB/core instead of 32 MB); the host
upcasts to f32 after gathering.  fp16 quantization error (~3e-4 rel) is far
below the softmax values' scale.

Per 128-row tile (per-engine pipeline):
  PE:  S = Q K^T into a 4-bank PSUM tile (4 x N=512 bf16 matmuls)
  DVE: fused PSUM->SBUF fp16 copy + row-max (tensor_scalar accum_out=max,
       1x mode: fp32 PSUM source), then reciprocal of max ONLY (the sum
       reciprocal is a separate op so exp never waits on the previous tile's
       accumulator read)
  ACT: exp(S * (1/max)) from the SBUF fp16 copy, fused row-sum accumulate
  DVE (mostly) / ACT: multiply by 1/rowsum into fp16 staging; DVE runs this
       at 4x (fp16 in/out SBUF), ACT at 1x -- pattern keeps engines balanced
  HWDGE DMA: 1 MB fp16 output chunks (2 row-tiles)
The MLP for batch 1 runs as a separate phase between the two S loops: S tiles
need all 8 PSUM banks (2 x 4-bank bufs), so interleaved MLP chunks would
steal a PSUM slot and bubble the PE->DVE pipeline.

x never touches the compute engines: a SWDGE cast-DMA produces a bf16 copy
of x in DRAM scratch, and a HWDGE xbar transpose-DMA loads x^T [D, M]
straight into SBUF.  This trims the serial phase-A ramp and removes all
DVE/PE x-prep work (casts, PE transposes, PSUM evacs).
"""

import numpy as np
import ml_dtypes

import concourse.bass as bass
import concourse.mybir as mybir
from concourse import bacc
import concourse.tile as tile
from concourse.bass import ts
from concourse.bass_utils import run_bass_kernel_spmd

F32 = mybir.dt.float32
BF16 = mybir.dt.bfloat16
FP16 = mybir.dt.float16
AF = mybir.ActivationFunctionType
ALU = mybir.AluOpType

N_CORES = 8
B, M, D, H, KF = 16, 2048, 128, 256, 64
BPC = B // N_CORES     # batches per core
MT = M // 128          # 16 row-tiles per batch
FC = M // 512          # 4 matmul free-chunks of 512
PAIR = 2               # row-tiles per output DMA (1 MB fp16 chunks)

# normalize engine per row-tile: DVE fp16->fp16 runs 4x (~0.59us/tile),
# ACT copy-with-scale is 1x (~1.9us/tile); ~11/16 on DVE balances the
# engines given ACT also owns the exp.
NORM_PATTERN = (
    "dve", "act", "dve", "dve", "act", "dve", "dve", "act",
    "dve", "dve", "act", "dve", "dve", "act", "dve", "dve",
)


def _evac_bias(nc, engine, out, in_, bias, relu):
    """out = [relu](in_ + bias), bias is [P,1] per-partition AP."""
    if engine == "act":
        nc.scalar.activation(
            out, in_, AF.Relu if relu else AF.Identity, bias=bias, scale=1.0
        )
    else:
        if relu:
            nc.vector.tensor_scalar(out, in_, bias, 0.0, op0=ALU.add, op1=ALU.max)
        else:
            nc.vector.tensor_scalar(out, in_, bias, None, op0=ALU.add)


def _norm(nc, engine, out, t, isum):
    if engine == "act":
        nc.scalar.mul(out, t, isum)
    else:
        nc.vector.tensor_scalar_mul(out, t, isum)


def build_nc():
    nc = bacc.Bacc()

    x = nc.dram_tensor("x", [BPC, M, D], F32, kind="ExternalInput")
    w1d, b1d, w2d, b2d = {}, {}, {}, {}
    for h in ("q", "k"):
        w1d[h] = nc.dram_tensor(f"{h}W1", [D, H], F32, kind="ExternalInput")
        b1d[h] = nc.dram_tensor(f"{h}b1", [H], F32, kind="ExternalInput")
        w2d[h] = nc.dram_tensor(f"{h}W2", [H, KF], F32, kind="ExternalInput")
        b2d[h] = nc.dram_tensor(f"{h}b2", [KF], F32, kind="ExternalInput")
    out = nc.dram_tensor("out", [BPC, M, M], FP16, kind="ExternalOutput")

    # [b, p, n, m]: out[b, n*128+p, m]
    out_r = out[:].rearrange("b (n p) m -> b p n m", p=128)

    with tile.TileContext(nc) as tc:
        with (
            tc.tile_pool(name="consts", bufs=1) as consts,
            tc.tile_pool(name="xstage", bufs=1, space="DRAM") as xstage,
            tc.tile_pool(name="xt", bufs=1) as xt_pool,
            tc.tile_pool(name="ht", bufs=2) as ht_pool,
            tc.tile_pool(name="qkt", bufs=2) as qkt_pool,
            tc.tile_pool(name="texp", bufs=4) as t_pool,
            tc.tile_pool(name="osb", bufs=4) as out_pool,
            tc.tile_pool(name="small", bufs=8) as small_pool,
            tc.tile_pool(name="psum", bufs=2, space="PSUM") as psum_pool,
        ):
            norm_i = 0

            # ---- weights first (tiny, gpsimd cast DMA), then x prep ----
            w1, w2, b1, b2 = {}, {}, {}, {}
            for h in ("q", "k"):
                w1[h] = consts.tile([D, H], BF16, tag=f"w1{h}", name=f"w1{h}")
                nc.gpsimd.dma_start(out=w1[h], in_=w1d[h][:])  # cast f32->bf16
                w2[h] = consts.tile([128, 2, KF], BF16, tag=f"w2{h}", name=f"w2{h}")
                nc.gpsimd.dma_start(
                    out=w2[h], in_=w2d[h][:].rearrange("(c p) k -> p c k", p=128)
                )
                b1[h] = consts.tile([128, 2], F32, tag=f"b1{h}", name=f"b1{h}")
                nc.sync.dma_start(
                    out=b1[h], in_=b1d[h][:].rearrange("(c p) -> p c", p=128)
                )
                b2[h] = consts.tile([KF, 1], F32, tag=f"b2{h}", name=f"b2{h}")
                nc.sync.dma_start(
                    out=b2[h], in_=b2d[h][:].rearrange("(k o) -> k o", o=1)
                )

            # ---- x path is pure DMA: cast f32->bf16 into DRAM scratch
            # (SWDGE), then xbar transpose-DMA straight into SBUF (HWDGE).
            xbf, xT = {}, {}
            for b in range(BPC):
                xbf[b] = xstage.tile([M, D], BF16, tag=f"xbf{b}", name="xbf")
                xT[b] = xt_pool.tile([128, M], BF16, tag=f"xt{b}", name="xT")
            for half in range(2):
                nc.gpsimd.dma_start(
                    out=xbf[0][ts(half, 1024), :], in_=x[0][ts(half, 1024), :]
                )
                nc.sync.dma_start_transpose(
                    out=xT[0][:, ts(half, 1024)], in_=xbf[0][ts(half, 1024), :]
                )
            for half in range(2):
                nc.gpsimd.dma_start(
                    out=xbf[1][ts(half, 1024), :], in_=x[1][ts(half, 1024), :]
                )
                nc.sync.dma_start_transpose(
                    out=xT[1][:, ts(half, 1024)], in_=xbf[1][ts(half, 1024), :]
                )

            def phase_a(b):
                """MLP for batch b: h = relu(x W1 + b1), qkt = h W2 + b2.
                Evacs split across ACT/DVE so both engines stay busy."""
                ht = {}
                for h in ("q", "k"):
                    ht[h] = ht_pool.tile([128, 2, M], BF16, tag=f"ht{h}", name=f"ht{h}")
                for h, pc, half in (
                    ("q", 0, 0), ("k", 0, 0), ("q", 0, 1), ("k", 0, 1),
                    ("q", 1, 0), ("k", 1, 0), ("q", 1, 1), ("k", 1, 1),
                ):
                    ps1 = psum_pool.tile([128, 1024], F32, tag="ps", name="ps1")
                    for fc in range(2):
                        nc.tensor.matmul(
                            ps1[:, ts(fc, 512)],
                            lhsT=w1[h][:, ts(pc, 128)],
                            rhs=xT[b][:, ts(half * 2 + fc, 512)],
                            start=True,
                            stop=True,
                        )
                    # split the evac across both engines
                    for e, fc in (("act", 0), ("dve", 1)):
                        _evac_bias(
                            nc,
                            e,
                            ht[h][:, pc, ts(half * 2 + fc, 512)],
                            ps1[:, ts(fc, 512)],
                            b1[h][:, pc : pc + 1],
                            relu=True,
                        )

                qkt = {}
                for h in ("q", "k"):
                    ps2 = psum_pool.tile([KF, M], F32, tag="ps", name="ps2")
                    for fc in range(FC):
                        for kc in range(2):
                            nc.tensor.matmul(
                                ps2[:, ts(fc, 512)],
                                lhsT=w2[h][:, kc, :],
                                rhs=ht[h][:, kc, ts(fc, 512)],
                                start=(kc == 0),
                                stop=(kc == 1),
                            )
                    q = qkt_pool.tile([KF, M], BF16, tag=f"qkt{h}", name=f"qkt{h}")
                    qkt[h] = q
                    for fc in range(FC):
                        _evac_bias(
                            nc,
                            ("act", "dve")[fc % 2],
                            q[:, ts(fc, 512)],
                            ps2[:, ts(fc, 512)],
                            b2[h],
                            relu=False,
                        )
                return qkt

            def s_loop(b, qkt):
                """S + softmax loop for batch b."""
                nonlocal norm_i
                osb_tiles = {}
                pending = None  # (rt, t_tile, sum_tile)

                def finish(j, t_j, isum_ap):
                    nonlocal norm_i
                    _norm(
                        nc,
                        NORM_PATTERN[norm_i % len(NORM_PATTERN)],
                        osb_tiles[j // PAIR][:, ts(j % PAIR, M)],
                        t_j,
                        isum_ap,
                    )
                    norm_i += 1
                    if j % PAIR == PAIR - 1:
                        osb = osb_tiles.pop(j // PAIR)
                        if j == MT - 1:
                            for jj in range(PAIR):
                                nc.sync.dma_start(
                                    out=out_r[b][:, j - PAIR + 1 + jj : j - PAIR + 2 + jj, :],
                                    in_=osb[:, ts(jj, M)],
                                )
                        else:
                            nc.sync.dma_start(
                                out=out_r[b][:, j - PAIR + 1 : j + 1, :],
                                in_=osb,
                            )

                for rt in range(MT):
                    ps_s = psum_pool.tile([128, M], F32, tag="ps", name="ps_s")
                    for fc in range(FC):
                        nc.tensor.matmul(
                            ps_s[:, ts(fc, 512)],
                            lhsT=qkt["q"][:, ts(rt, 128)],
                            rhs=qkt["k"][:, ts(fc, 512)],
                            start=True,
                            stop=True,
                        )
                    # Evacuate S from PSUM to fp16 SBUF with fused row-max;
                    # frees the PSUM slot so exp reads the SBUF copy.
                    sc_t = t_pool.tile([128, M], FP16, tag="sc", name="sc")
                    mx = small_pool.tile([128, 1], F32, tag="mx", name="mx")
                    nc.vector.tensor_scalar(
                        sc_t,
                        ps_s,
                        0.0,
                        None,
                        op0=ALU.add,
                        op1=ALU.max,
                        accum_out=mx,
                    )
                    # reciprocal of max only -- exp depends on nothing else
                    imax = small_pool.tile([128, 1], F32, tag="im", name="imax")
                    nc.vector.reciprocal(imax, mx)

                    sum_t = small_pool.tile([128, 1], F32, tag="sm", name="sum")
                    t_t = t_pool.tile([128, M], FP16, tag="t")
                    nc.scalar.activation(
                        t_t,
                        sc_t,
                        AF.Exp,
                        bias=0.0,
                        scale=imax,
                        accum_out=sum_t,
                    )

                    if rt % PAIR == 0:
                        osb_tiles[rt // PAIR] = out_pool.tile(
                            [128, PAIR * M], FP16, tag="o", name="osb"
                        )
                    if pending is not None:
                        pj, pt, psum_t = pending
                        isum = small_pool.tile([128, 1], F32, tag="is", name="isum")
                        nc.vector.reciprocal(isum, psum_t)
                        finish(pj, pt, isum)
                    pending = (rt, t_t, sum_t)

                pj, pt, psum_t = pending
                isum = small_pool.tile([128, 1], F32, tag="is", name="isum")
                nc.vector.reciprocal(isum, psum_t)
                finish(pj, pt, isum)

            qkt0 = phase_a(0)
            s_loop(0, qkt0)
            qkt1 = phase_a(1)
            s_loop(1, qkt1)
    nc.finalize()
    return nc


_NC_CACHE = None


def _get_nc():
    global _NC_CACHE
    if _NC_CACHE is None:
        _NC_CACHE = build_nc()
    return _NC_CACHE


def run(inputs, trace=False, trace_cores=None):
    """Run on 8 cores; returns (full_output [B,M,M] f32, BassKernelResults)."""
    nc = _get_nc()
    in_maps = []
    x = np.ascontiguousarray(inputs["x"], dtype=np.float32)
    for c in range(N_CORES):
        im = {"x": np.ascontiguousarray(x[c * BPC : (c + 1) * BPC])}
        for k in ("qW1", "qb1", "qW2", "qb2", "kW1", "kb1", "kW2", "kb2"):
            im[k] = np.ascontiguousarray(inputs[k], dtype=np.float32)
        in_maps.append(im)
    res = run_bass_kernel_spmd(
        nc,
        in_maps,
        core_ids=list(range(N_CORES)),
        trace=trace,
        trace_cores=trace_cores,
    )
    outs = [np.asarray(r["out"]) for r in res.results]
    full = np.concatenate(outs, axis=0).astype(np.float32)
    assert full.shape == (B, M, M) and full.dtype == np.float32
    return full, res


def kernel(**inputs) -> np.ndarray:
    out, _ = run(inputs, trace=False)
    return out


# revision 6
# speedup vs baseline: 1.0851x; 1.0851x over previous
"""Self-contained Trainium2 Bass kernel for nn_CA_9363028705415 (sparse_attention).

Computes, per batch b:
    Q = relu(x[b] @ qW1 + qb1) @ qW2 + qb2          # [M, K]
    Kt = relu(x[b] @ kW1 + kb1) @ kW2 + kb2         # [M, K]
    S = Q @ Kt.T                                    # [M, M]
    out[b] = softmax(S / rowmax(S), axis=-1)        # max-DIVISION normalization

Shapes: B=16, M=2048, D=128, H=256, K=64.  Output [16, 2048, 2048] f32 (256 MB).

Sharding: data-parallel over batch across 8 NeuronCores; 2 batches/core; tiny
MLP weights replicated.  Single NEFF run SPMD via run_bass_kernel_spmd.

Device writes the output in fp16 (16 MB/core instead of 32 MB); the host
upcasts to f32 after gathering.  fp16 quantization error (~3e-4 rel) is far
below the 2e-2 gate.

x never touches the compute engines: a SWDGE cast-DMA produces a bf16 copy of
each x token-half in DRAM scratch, and a HWDGE xbar transpose-DMA loads
x^T [D, M] straight into SBUF (per-half tiles so the casts/transposes/mlp
pipeline at half granularity).

S is computed in two [128, 1024] PSUM half-tiles (2 banks each; psum_s pool
2 bufs = 4 banks) so the OTHER 4 banks serve a dedicated MLP pool: batch 1's
MLP chunks interleave into batch 0's S loop without stealing S-pipeline slots.

Per 128-row tile:
  PE:  2x2 matmuls -> two [128,1024] f32 PSUM halves
  DVE: per half, fused PSUM->SBUF fp16 copy + running row-max
       (tensor_scalar accum_out=max, 1x mode: fp32 PSUM source);
       reduce_max over the two half-maxes; reciprocal of max ONLY
  ACT: exp(S * (1/max)) over the full fp16 row, fused row-sum accumulate
  DVE: reciprocal of previous tile's sum (separate op so exp never waits on
       the previous accumulator read), then the previous tile's normalize
       multiply at 4x (fp16 in/out SBUF); some norms go to ACT (1x) to
       balance -- NORM_PATTERN
  HWDGE DMA: 1 MB fp16 output chunks (2 row-tiles; final tile split for tail)
"""

import numpy as np

import concourse.bass as bass
import concourse.mybir as mybir
from concourse import bacc
import concourse.tile as tile
from concourse.bass import ts
from concourse.bass_utils import run_bass_kernel_spmd

F32 = mybir.dt.float32
BF16 = mybir.dt.bfloat16
FP16 = mybir.dt.float16
AF = mybir.ActivationFunctionType
ALU = mybir.AluOpType

N_CORES = 8
B, M, D, H, KF = 16, 2048, 128, 256, 64
BPC = B // N_CORES     # batches per core
MT = M // 128          # 16 row-tiles per batch
HM = M // 2            # 1024: half-tile free size
PAIR = 2               # row-tiles per output DMA (1 MB fp16 chunks)

# normalize engine per row-tile: DVE fp16->fp16 runs 4x (~0.6us/tile),
# ACT copy-with-scale is 1x (~2us/tile); ~11/16 on DVE balances the
# engines given ACT also owns the exp.
NORM_PATTERN = (
    "dve", "act", "dve", "dve", "act", "dve", "dve", "act",
    "dve", "dve", "act", "dve", "dve", "act", "dve", "dve",
)


def _evac_bias(nc, engine, out, in_, bias, relu):
    """out = [relu](in_ + bias), bias is [P,1] per-partition AP."""
    if engine == "act":
        nc.scalar.activation(
            out, in_, AF.Relu if relu else AF.Identity, bias=bias, scale=1.0
        )
    else:
        if relu:
            nc.vector.tensor_scalar(out, in_, bias, 0.0, op0=ALU.add, op1=ALU.max)
        else:
            nc.vector.tensor_scalar(out, in_, bias, None, op0=ALU.add)


def _norm(nc, engine, out, t, isum):
    if engine == "act":
        nc.scalar.mul(out, t, isum)
    else:
        nc.vector.tensor_scalar_mul(out, t, isum)


def build_nc():
    nc = bacc.Bacc()

    x = nc.dram_tensor("x", [BPC, M, D], F32, kind="ExternalInput")
    w1d, b1d, w2d, b2d = {}, {}, {}, {}
    for h in ("q", "k"):
        w1d[h] = nc.dram_tensor(f"{h}W1", [D, H], F32, kind="ExternalInput")
        b1d[h] = nc.dram_tensor(f"{h}b1", [H], F32, kind="ExternalInput")
        w2d[h] = nc.dram_tensor(f"{h}W2", [H, KF], F32, kind="ExternalInput")
        b2d[h] = nc.dram_tensor(f"{h}b2", [KF], F32, kind="ExternalInput")
    out = nc.dram_tensor("out", [BPC, M, M], FP16, kind="ExternalOutput")

    # [b, p, n, m]: out[b, n*128+p, m]
    out_r = out[:].rearrange("b (n p) m -> b p n m", p=128)

    with tile.TileContext(nc) as tc:
        with (
            tc.tile_pool(name="consts", bufs=1) as consts,
            tc.tile_pool(name="xstage", bufs=1, space="DRAM") as xstage,
            tc.tile_pool(name="xt", bufs=1) as xt_pool,
            tc.tile_pool(name="ht", bufs=2) as ht_pool,
            tc.tile_pool(name="qkt", bufs=2) as qkt_pool,
            tc.tile_pool(name="texp", bufs=4) as t_pool,
            tc.tile_pool(name="osb", bufs=4) as out_pool,
            tc.tile_pool(name="small", bufs=8) as small_pool,
            tc.tile_pool(name="psum_s", bufs=2, space="PSUM") as psum_s,
            tc.tile_pool(name="psum_mlp", bufs=2, space="PSUM") as psum_mlp,
        ):
            norm_i = 0

            # ---- x casts start immediately (SWDGE queue head) ----
            xbf, xT = {}, {}
            for b in range(BPC):
                for half in range(2):
                    xbf[b, half] = xstage.tile(
                        [HM, D], BF16, tag=f"xbf{b}{half}", name="xbf"
                    )
                    xT[b, half] = xt_pool.tile(
                        [128, HM], BF16, tag=f"xt{b}{half}", name="xT"
                    )
            for b in range(BPC):
                for half in range(2):
                    nc.gpsimd.dma_start(
                        out=xbf[b, half], in_=x[b][ts(half, HM), :]
                    )
                    nc.sync.dma_start_transpose(out=xT[b, half], in_=xbf[b, half])

            # ---- weights via HWDGE f32 loads + tiny engine casts ----
            w1, w2, b1, b2 = {}, {}, {}, {}
            wraw = {}
            for h in ("q", "k"):
                wraw[h, 1] = consts.tile([D, H], F32, tag=f"w1r{h}", name=f"w1r{h}")
                nc.sync.dma_start(out=wraw[h, 1], in_=w1d[h][:])
                wraw[h, 2] = consts.tile(
                    [128, 2, KF], F32, tag=f"w2r{h}", name=f"w2r{h}"
                )
                nc.sync.dma_start(
                    out=wraw[h, 2], in_=w2d[h][:].rearrange("(c p) k -> p c k", p=128)
                )
                b1[h] = consts.tile([128, 2], F32, tag=f"b1{h}", name=f"b1{h}")
                nc.sync.dma_start(
                    out=b1[h], in_=b1d[h][:].rearrange("(c p) -> p c", p=128)
                )
                b2[h] = consts.tile([KF, 1], F32, tag=f"b2{h}", name=f"b2{h}")
                nc.sync.dma_start(
                    out=b2[h], in_=b2d[h][:].rearrange("(k o) -> k o", o=1)
                )
            for h in ("q", "k"):
                w1[h] = consts.tile([D, H], BF16, tag=f"w1{h}", name=f"w1{h}")
                nc.vector.tensor_copy(w1[h], wraw[h, 1])
                w2[h] = consts.tile([128, 2, KF], BF16, tag=f"w2{h}", name=f"w2{h}")
                nc.vector.tensor_copy(w2[h], wraw[h, 2])

            # ---- PE warm-up: ~8 dummy matmuls trip the HAM clock gate to
            # 2.4 GHz before the real MLP starts ----
            wu = consts.tile([128, 512], BF16, tag="wu", name="warm")
            nc.vector.memset(wu, 0.0)
            for i in range(8):
                ps_w = psum_s.tile([128, 512], F32, tag="ps", name="ps_warm")
                nc.tensor.matmul(
                    ps_w, lhsT=wu[:, 0:128], rhs=wu, start=True, stop=True
                )

            def phase_a_chunks(b):
                """MLP chunk closures for batch b (fine-grained so they can
                interleave into the previous batch's S loop).  Each chunk uses
                one psum_mlp slot (2 banks)."""
                ht = {}
                for h in ("q", "k"):
                    ht[h] = ht_pool.tile(
                        [128, 2, M], BF16, tag=f"ht{h}", name=f"ht{h}"
                    )
                qkt = {}
                for h in ("q", "k"):
                    qkt[h] = qkt_pool.tile(
                        [KF, M], BF16, tag=f"qkt{h}", name=f"qkt{h}"
                    )

                def c_mlp1(h, pc, half):
                    def go():
                        ps1 = psum_mlp.tile([128, HM], F32, tag="ps", name="ps1")
                        for fc in range(2):
                            nc.tensor.matmul(
                                ps1[:, ts(fc, 512)],
                                lhsT=w1[h][:, ts(pc, 128)],
                                rhs=xT[b, half][:, ts(fc, 512)],
                                start=True,
                                stop=True,
                            )
                        for e, fc in (("act", 0), ("dve", 1)):
                            _evac_bias(
                                nc,
                                e,
                                ht[h][:, pc, ts(half * 2 + fc, 512)],
                                ps1[:, ts(fc, 512)],
                                b1[h][:, pc : pc + 1],
                                relu=True,
                            )
                    return go

                def c_mlp2(h, mh):
                    def go():
                        ps2 = psum_mlp.tile([KF, HM], F32, tag="ps", name="ps2")
                        for fc in range(2):
                            for kc in range(2):
                                nc.tensor.matmul(
                                    ps2[:, ts(fc, 512)],
                                    lhsT=w2[h][:, kc, :],
                                    rhs=ht[h][:, kc, ts(mh * 2 + fc, 512)],
                                    start=(kc == 0),
                                    stop=(kc == 1),
                                )
                        for e, fc in (("act", 0), ("dve", 1)):
                            _evac_bias(
                                nc,
                                e,
                                qkt[h][:, ts(mh * 2 + fc, 512)],
                                ps2[:, ts(fc, 512)],
                                b2[h],
                                relu=False,
                            )
                    return go

                chunks = []
                for half in range(2):
                    for h, pc in (("q", 0), ("k", 0), ("q", 1), ("k", 1)):
                        chunks.append(c_mlp1(h, pc, half))
                for mh in range(2):
                    for h in ("q", "k"):
                        chunks.append(c_mlp2(h, mh))
                return qkt, chunks

            def s_loop(b, qkt, next_chunks):
                """S + softmax loop for batch b, interleaving next_chunks
                (next batch's MLP) into the iterations."""
                nonlocal norm_i
                osb_tiles = {}
                pending = None  # (rt, t_tile, sum_tile)

                def finish(j, t_j, isum_ap):
                    nonlocal norm_i
                    _norm(
                        nc,
                        NORM_PATTERN[norm_i % len(NORM_PATTERN)],
                        osb_tiles[j // PAIR][:, ts(j % PAIR, M)],
                        t_j,
                        isum_ap,
                    )
                    norm_i += 1
                    if j % PAIR == PAIR - 1:
                        osb = osb_tiles.pop(j // PAIR)
                        if j == MT - 1:
                            # tail: per-tile, last tile split in half
                            nc.sync.dma_start(
                                out=out_r[b][:, j - 1 : j, :], in_=osb[:, 0:M]
                            )
                            for hh in range(2):
                                nc.sync.dma_start(
                                    out=out_r[b][:, j : j + 1, ts(hh, HM)],
                                    in_=osb[:, M + hh * HM : M + (hh + 1) * HM],
                                )
                        else:
                            nc.sync.dma_start(
                                out=out_r[b][:, j - PAIR + 1 : j + 1, :],
                                in_=osb,
                            )

                for rt in range(MT):
                    sc_t = t_pool.tile([128, M], FP16, tag="sc", name="sc")
                    mx2 = small_pool.tile([128, 2], F32, tag="mx", name="mx2")
                    for hf in range(2):
                        ps_s = psum_s.tile([128, HM], F32, tag="ps", name="ps_s")
                        for fc in range(2):
                            nc.tensor.matmul(
                                ps_s[:, ts(fc, 512)],
                                lhsT=qkt["q"][:, ts(rt, 128)],
                                rhs=qkt["k"][:, ts(hf * 2 + fc, 512)],
                                start=True,
                                stop=True,
                            )
                        # fused PSUM->SBUF fp16 evac + row-max of this half
                        nc.vector.tensor_scalar(
                            sc_t[:, ts(hf, HM)],
                            ps_s,
                            0.0,
                            None,
                            op0=ALU.add,
                            op1=ALU.max,
                            accum_out=mx2[:, hf : hf + 1],
                        )
                    mx = small_pool.tile([128, 1], F32, tag="m1", name="mx")
                    nc.vector.reduce_max(mx, mx2, axis=mybir.AxisListType.X)
                    imax = small_pool.tile([128, 1], F32, tag="im", name="imax")
                    nc.vector.reciprocal(imax, mx)

                    sum_t = small_pool.tile([128, 1], F32, tag="sm", name="sum")
                    t_t = t_pool.tile([128, M], FP16, tag="t")
                    nc.scalar.activation(
                        t_t,
                        sc_t,
                        AF.Exp,
                        bias=0.0,
                        scale=imax,
                        accum_out=sum_t,
                    )

                    if rt % PAIR == 0:
                        osb_tiles[rt // PAIR] = out_pool.tile(
                            [128, PAIR * M], FP16, tag="o", name="osb"
                        )
                    if pending is not None:
                        pj, pt, psum_t = pending
                        isum = small_pool.tile([128, 1], F32, tag="is", name="isum")
                        nc.vector.reciprocal(isum, psum_t)
                        finish(pj, pt, isum)
                    pending = (rt, t_t, sum_t)

                    if next_chunks:
                        next_chunks.pop(0)()

                pj, pt, psum_t = pending
                isum = small_pool.tile([128, 1], F32, tag="is", name="isum")
                nc.vector.reciprocal(isum, psum_t)
                finish(pj, pt, isum)
                while next_chunks:
                    next_chunks.pop(0)()

            qkt0, chunks0 = phase_a_chunks(0)
            for c in chunks0:
                c()
            qkt1, chunks1 = phase_a_chunks(1)
            s_loop(0, qkt0, chunks1)
            s_loop(1, qkt1, [])
    nc.finalize()
    return nc


_NC_CACHE = None


def _get_nc():
    global _NC_CACHE
    if _NC_CACHE is None:
        _NC_CACHE = build_nc()
    return _NC_CACHE


def run(inputs, trace=False, trace_cores=None):
    """Run on 8 cores; returns (full_output [B,M,M] f32, BassKernelResults)."""
    nc = _get_nc()
    in_maps = []
    x = np.ascontiguousarray(inputs["x"], dtype=np.float32)
    for c in range(N_CORES):
        im = {"x": np.ascontiguousarray(x[c * BPC : (c + 1) * BPC])}
        for k in ("qW1", "qb1", "qW2", "qb2", "kW1", "kb1", "kW2", "kb2"):
            im[k] = np.ascontiguousarray(inputs[k], dtype=np.float32)
        in_maps.append(im)
    res = run_bass_kernel_spmd(
        nc,
        in_maps,
        core_ids=list(range(N_CORES)),
        trace=trace,
        trace_cores=trace_cores,
    )
    outs = [np.asarray(r["out"]) for r in res.results]
    full = np.concatenate(outs, axis=0).astype(np.float32)
    assert full.shape == (B, M, M) and full.dtype == np.float32
    return full, res


def kernel(**inputs) -> np.ndarray:
    out, _ = run(inputs, trace=False)
    return out


# revision 13
# speedup vs baseline: 1.1383x; 1.0489x over previous
"""Self-contained Trainium2 Bass kernel for nn_CA_9363028705415 (sparse_attention).

Computes, per batch b:
    Q = relu(x[b] @ qW1 + qb1) @ qW2 + qb2          # [M, K]
    Kt = relu(x[b] @ kW1 + kb1) @ kW2 + kb2         # [M, K]
    S = Q @ Kt.T                                    # [M, M]
    out[b] = softmax(S / rowmax(S), axis=-1)        # max-DIVISION normalization

Shapes: B=16, M=2048, D=128, H=256, K=64.  Output [16, 2048, 2048] f32 (256 MB).

Sharding: data-parallel over batch across 8 NeuronCores; 2 batches/core; tiny
MLP weights replicated.  Single NEFF run SPMD via run_bass_kernel_spmd.

Device writes the output in fp16 (16 MB/core instead of 32 MB); the host
upcasts to f32 after gathering.  fp16 quantization error (~3e-4 rel) is far
below the 2e-2 gate.

x never touches the compute engines: a SWDGE cast-DMA produces a bf16 copy of
each x token-half in DRAM scratch, and a HWDGE xbar transpose-DMA loads
x^T [D, M] straight into SBUF (per-half tiles so the casts/transposes/mlp
pipeline at half granularity).

S is computed in two [128, 1024] PSUM half-tiles (2 banks each; psum_s pool
2 bufs = 4 banks) so the OTHER 4 banks serve a dedicated MLP pool: batch 1's
MLP chunks interleave into batch 0's S loop without stealing S-pipeline slots.

Per 128-row tile:
  PE:  2x2 matmuls -> two [128,1024] f32 PSUM halves
  DVE: per half, fused PSUM->SBUF fp16 copy + running row-max
       (tensor_scalar accum_out=max, 1x mode: fp32 PSUM source);
       reduce_max over the two half-maxes; reciprocal of max ONLY
  ACT: exp(S * (1/max)) over the full fp16 row, fused row-sum accumulate
  DVE: reciprocal of previous tile's sum (separate op so exp never waits on
       the previous accumulator read), then the previous tile's normalize
       multiply at 4x (fp16 in/out SBUF); some norms go to ACT (1x) to
       balance -- NORM_PATTERN
  HWDGE DMA: 1 MB fp16 output chunks (2 row-tiles; final tile split for tail)
"""

import numpy as np
import ml_dtypes

import concourse.bass as bass
import concourse.mybir as mybir
from concourse import bacc
import concourse.tile as tile
from concourse.bass import ts
from concourse.bass_utils import run_bass_kernel_spmd

F32 = mybir.dt.float32
BF16 = mybir.dt.bfloat16
FP16 = mybir.dt.float16
AF = mybir.ActivationFunctionType
ALU = mybir.AluOpType

N_CORES = 8
B, M, D, H, KF = 16, 2048, 128, 256, 64
BPC = B // N_CORES     # batches per core
MT = M // 128          # 16 row-tiles per batch
HM = M // 2            # 1024: half-tile free size

# normalize engine per row-tile: DVE fp16->fp16 runs 4x (~0.6us/tile),
# ACT copy-with-scale is 1x (~2us/tile); ~11/16 on DVE balances the
# engines given ACT also owns the exp.
NORM_PATTERN = (
    "dve", "act", "dve", "dve", "act", "dve", "dve", "act",
    "dve", "dve", "act", "dve", "dve", "act", "dve", "dve",
)


def _evac_bias(nc, engine, out, in_, bias, relu):
    """out = [relu](in_ + bias), bias is [P,1] per-partition AP."""
    if engine == "act":
        nc.scalar.activation(
            out, in_, AF.Relu if relu else AF.Identity, bias=bias, scale=1.0
        )
    else:
        if relu:
            nc.vector.tensor_scalar(out, in_, bias, 0.0, op0=ALU.add, op1=ALU.max)
        else:
            nc.vector.tensor_scalar(out, in_, bias, None, op0=ALU.add)


def _norm(nc, engine, out, t, isum):
    if engine == "act":
        nc.scalar.mul(out, t, isum)
    else:
        nc.vector.tensor_scalar_mul(out, t, isum)


def build_nc():
    nc = bacc.Bacc()

    x = nc.dram_tensor("x", [BPC, M, D], F32, kind="ExternalInput")
    w1d, b1d, w2d, b2d = {}, {}, {}, {}
    for h in ("q", "k"):
        w1d[h] = nc.dram_tensor(f"{h}W1", [D, H], F32, kind="ExternalInput")
        b1d[h] = nc.dram_tensor(f"{h}b1", [H], F32, kind="ExternalInput")
        w2d[h] = nc.dram_tensor(f"{h}W2", [H, KF], F32, kind="ExternalInput")
        b2d[h] = nc.dram_tensor(f"{h}b2", [KF], F32, kind="ExternalInput")
    out = nc.dram_tensor("out", [BPC, M, M], FP16, kind="ExternalOutput")

    ident_np = np.eye(128, dtype=ml_dtypes.bfloat16)
    ident_dram = nc.inline_tensor(ident_np, name="ident_data")

    # [p, n, d]: batch-0 token (n*128+p), feature d
    x0_r = x[0].rearrange("(n p) d -> p n d", p=128)
    # [b, p, n, m]: out[b, n*128+p, m]
    out_r = out[:].rearrange("b (n p) m -> b p n m", p=128)

    with tile.TileContext(nc) as tc:
        with (
            tc.tile_pool(name="consts", bufs=1) as consts,
            tc.tile_pool(name="xstage", bufs=1, space="DRAM") as xstage,
            tc.tile_pool(name="xin", bufs=1) as xin_pool,
            tc.tile_pool(name="xt", bufs=1) as xt_pool,
            tc.tile_pool(name="ht", bufs=2) as ht_pool,
            tc.tile_pool(name="qkt", bufs=2) as qkt_pool,
            tc.tile_pool(name="texp", bufs=4) as t_pool,
            tc.tile_pool(name="osb", bufs=6) as out_pool,
            tc.tile_pool(name="small", bufs=8) as small_pool,
            tc.tile_pool(name="psum_s", bufs=2, space="PSUM") as psum_s,
            tc.tile_pool(name="psum_mlp", bufs=2, space="PSUM") as psum_mlp,
        ):
            norm_i = 0

            # ---- batch-1 x casts start immediately (SWDGE queue is
            # otherwise idle); batch 0 goes the fast in-SBUF route ----
            xbf, xT1 = {}, {}
            for half in range(2):
                xbf[half] = xstage.tile([HM, D], BF16, tag=f"xbf{half}", name="xbf")
                xT1[half] = xt_pool.tile([128, HM], BF16, tag=f"xt1{half}", name="xT1")
                nc.gpsimd.dma_start(out=xbf[half], in_=x[1][ts(half, HM), :])

            # ---- batch-0 x f32 halves (head of the sync DMA queue) ----
            xf = {}
            for half in range(2):
                xf[half] = xin_pool.tile([128, 8, 128], F32, tag=f"xf{half}", name="xf")
                nc.sync.dma_start(
                    out=xf[half], in_=x0_r[:, half * 8 : (half + 1) * 8, :]
                )

            # ---- weights/biases via HWDGE f32 loads + tiny engine casts ----
            ident = consts.tile([128, 128], BF16, tag="ident")
            nc.sync.dma_start(out=ident, in_=ident_dram[:])
            w1, w2, b1, b2 = {}, {}, {}, {}
            wraw = {}
            for h in ("q", "k"):
                wraw[h, 1] = consts.tile([D, H], F32, tag=f"w1r{h}", name=f"w1r{h}")
                nc.sync.dma_start(out=wraw[h, 1], in_=w1d[h][:])
                wraw[h, 2] = consts.tile(
                    [128, 2, KF], F32, tag=f"w2r{h}", name=f"w2r{h}"
                )
                nc.sync.dma_start(
                    out=wraw[h, 2], in_=w2d[h][:].rearrange("(c p) k -> p c k", p=128)
                )
                b1[h] = consts.tile([128, 2], F32, tag=f"b1{h}", name=f"b1{h}")
                nc.sync.dma_start(
                    out=b1[h], in_=b1d[h][:].rearrange("(c p) -> p c", p=128)
                )
                b2[h] = consts.tile([KF, 1], F32, tag=f"b2{h}", name=f"b2{h}")
                nc.sync.dma_start(
                    out=b2[h], in_=b2d[h][:].rearrange("(k o) -> k o", o=1)
                )
            # batch-1 transpose-DMAs (sync queue, after the small loads)
            for half in range(2):
                nc.sync.dma_start_transpose(out=xT1[half], in_=xbf[half])
            for h in ("q", "k"):
                w1[h] = consts.tile([D, H], BF16, tag=f"w1{h}", name=f"w1{h}")
                nc.vector.tensor_copy(w1[h], wraw[h, 1])
                w2[h] = consts.tile([128, 2, KF], BF16, tag=f"w2{h}", name=f"w2{h}")
                nc.vector.tensor_copy(w2[h], wraw[h, 2])

            # ---- PE warm-up: dummy matmuls trip the HAM clock gate to
            # 2.4 GHz before the real MLP starts ----
            wu = consts.tile([128, 512], BF16, tag="wu", name="warm")
            nc.vector.memset(wu, 0.0)
            for i in range(8):
                ps_w = psum_s.tile([128, 512], F32, tag="ps", name="ps_warm")
                nc.tensor.matmul(
                    ps_w, lhsT=wu[:, 0:128], rhs=wu, start=True, stop=True
                )

            # ---- batch-0 xT: DVE cast + warm-PE transposes, per half ----
            xT0 = xt_pool.tile([128, M], BF16, tag="xt0", name="xT0")
            xsb = xin_pool.tile([128, 2, 8, 128], BF16, tag="xsb", name="xsb")

            def b0_prep(half):
                nc.vector.tensor_copy(xsb[:, half], xf[half])
                tp = psum_mlp.tile([128, HM], BF16, tag="ps", name="tp")
                for it in range(8):
                    nc.tensor.transpose(
                        tp[:, ts(it, 128)], xsb[:, half, it, :], ident
                    )
                for e, fc in (("act", 0), ("dve", 1)):
                    dst = xT0[:, half * HM + fc * 512 : half * HM + (fc + 1) * 512]
                    if e == "act":
                        nc.scalar.copy(dst, tp[:, ts(fc, 512)])
                    else:
                        nc.vector.tensor_copy(dst, tp[:, ts(fc, 512)])

            def phase_a_chunks(b):
                """MLP chunk closures for batch b (fine-grained so they can
                interleave into the previous batch's S loop).  Each chunk uses
                one psum_mlp slot (2 banks)."""
                if b == 0:
                    xT_ap = lambda half, fc: xT0[:, half * HM + fc * 512 : half * HM + (fc + 1) * 512]
                else:
                    xT_ap = lambda half, fc: xT1[half][:, ts(fc, 512)]
                ht = {}
                for h in ("q", "k"):
                    ht[h] = ht_pool.tile(
                        [128, 2, M], BF16, tag=f"ht{h}", name=f"ht{h}"
                    )
                qkt = {}
                for h in ("q", "k"):
                    qkt[h] = qkt_pool.tile(
                        [KF, M], BF16, tag=f"qkt{h}", name=f"qkt{h}"
                    )

                def c_mlp1(h, pc, half):
                    def go():
                        ps1 = psum_mlp.tile([128, HM], F32, tag="ps", name="ps1")
                        for fc in range(2):
                            nc.tensor.matmul(
                                ps1[:, ts(fc, 512)],
                                lhsT=w1[h][:, ts(pc, 128)],
                                rhs=xT_ap(half, fc),
                                start=True,
                                stop=True,
                            )
                        for e, fc in (("act", 0), ("dve", 1)):
                            _evac_bias(
                                nc,
                                e,
                                ht[h][:, pc, ts(half * 2 + fc, 512)],
                                ps1[:, ts(fc, 512)],
                                b1[h][:, pc : pc + 1],
                                relu=True,
                            )
                    return go

                def c_mlp2(h, mh):
                    def go():
                        ps2 = psum_mlp.tile([KF, HM], F32, tag="ps", name="ps2")
                        for fc in range(2):
                            for kc in range(2):
                                nc.tensor.matmul(
                                    ps2[:, ts(fc, 512)],
                                    lhsT=w2[h][:, kc, :],
                                    rhs=ht[h][:, kc, ts(mh * 2 + fc, 512)],
                                    start=(kc == 0),
                                    stop=(kc == 1),
                                )
                        for e, fc in (("act", 0), ("dve", 1)):
                            _evac_bias(
                                nc,
                                e,
                                qkt[h][:, ts(mh * 2 + fc, 512)],
                                ps2[:, ts(fc, 512)],
                                b2[h],
                                relu=False,
                            )
                    return go

                chunks = []
                for half in range(2):
                    for h, pc in (("q", 0), ("k", 0), ("q", 1), ("k", 1)):
                        chunks.append(c_mlp1(h, pc, half))
                for mh in range(2):
                    for h in ("q", "k"):
                        chunks.append(c_mlp2(h, mh))
                return qkt, chunks

            def s_loop(b, qkt, next_chunks):
                """S + softmax loop for batch b, interleaving next_chunks
                (next batch's MLP) into the iterations."""
                nonlocal norm_i
                pending = None  # (rt, t_tile, sum_tile)

                def finish(j, t_j, isum_ap):
                    nonlocal norm_i
                    osb = out_pool.tile([128, M], FP16, tag="o", name="osb")
                    _norm(
                        nc,
                        NORM_PATTERN[norm_i % len(NORM_PATTERN)],
                        osb,
                        t_j,
                        isum_ap,
                    )
                    norm_i += 1
                    if j == MT - 1:
                        # tail: split the last tile's DMA in half
                        for hh in range(2):
                            nc.sync.dma_start(
                                out=out_r[b][:, j : j + 1, ts(hh, HM)],
                                in_=osb[:, ts(hh, HM)],
                            )
                    else:
                        nc.sync.dma_start(out=out_r[b][:, j : j + 1, :], in_=osb)

                for rt in range(MT):
                    sc_t = t_pool.tile([128, M], FP16, tag="sc", name="sc")
                    mx2 = small_pool.tile([128, 2], F32, tag="mx", name="mx2")
                    for hf in range(2):
                        ps_s = psum_s.tile([128, HM], F32, tag="ps", name="ps_s")
                        for fc in range(2):
                            nc.tensor.matmul(
                                ps_s[:, ts(fc, 512)],
                                lhsT=qkt["q"][:, ts(rt, 128)],
                                rhs=qkt["k"][:, ts(hf * 2 + fc, 512)],
                                start=True,
                                stop=True,
                            )
                        # fused PSUM->SBUF fp16 evac + row-max of this half
                        nc.vector.tensor_scalar(
                            sc_t[:, ts(hf, HM)],
                            ps_s,
                            0.0,
                            None,
                            op0=ALU.add,
                            op1=ALU.max,
                            accum_out=mx2[:, hf : hf + 1],
                        )
                    mx = small_pool.tile([128, 1], F32, tag="m1", name="mx")
                    nc.vector.reduce_max(mx, mx2, axis=mybir.AxisListType.X)
                    imax = small_pool.tile([128, 1], F32, tag="im", name="imax")
                    nc.vector.reciprocal(imax, mx)

                    sum_t = small_pool.tile([128, 1], F32, tag="sm", name="sum")
                    t_t = t_pool.tile([128, M], FP16, tag="t")
                    nc.scalar.activation(
                        t_t,
                        sc_t,
                        AF.Exp,
                        bias=0.0,
                        scale=imax,
                        accum_out=sum_t,
                    )

                    if pending is not None:
                        pj, pt, psum_t = pending
                        isum = small_pool.tile([128, 1], F32, tag="is", name="isum")
                        nc.vector.reciprocal(isum, psum_t)
                        finish(pj, pt, isum)
                    pending = (rt, t_t, sum_t)

                    if next_chunks:
                        next_chunks.pop(0)()

                pj, pt, psum_t = pending
                isum = small_pool.tile([128, 1], F32, tag="is", name="isum")
                nc.vector.reciprocal(isum, psum_t)
                finish(pj, pt, isum)
                while next_chunks:
                    next_chunks.pop(0)()

            qkt0, chunks0 = phase_a_chunks(0)
            # A0: prep half 0, its mlp1 chunks, prep half 1, the rest
            b0_prep(0)
            for c in chunks0[0:4]:
                c()
            b0_prep(1)
            for c in chunks0[4:]:
                c()
            qkt1, chunks1 = phase_a_chunks(1)
            s_loop(0, qkt0, chunks1)
            s_loop(1, qkt1, [])
    nc.finalize()
    return nc


_NC_CACHE = None


def _get_nc():
    global _NC_CACHE
    if _NC_CACHE is None:
        _NC_CACHE = build_nc()
    return _NC_CACHE


def run(inputs, trace=False, trace_cores=None):
    """Run on 8 cores; returns (full_output [B,M,M] f32, BassKernelResults)."""
    nc = _get_nc()
    in_maps = []
    x = np.ascontiguousarray(inputs["x"], dtype=np.float32)
    for c in range(N_CORES):
        im = {"x": np.ascontiguousarray(x[c * BPC : (c + 1) * BPC])}
        for k in ("qW1", "qb1", "qW2", "qb2", "kW1", "kb1", "kW2", "kb2"):
            im[k] = np.ascontiguousarray(inputs[k], dtype=np.float32)
        in_maps.append(im)
    res = run_bass_kernel_spmd(
        nc,
        in_maps,
        core_ids=list(range(N_CORES)),
        trace=trace,
        trace_cores=trace_cores,
    )
    outs = [np.asarray(r["out"]) for r in res.results]
    full = np.concatenate(outs, axis=0).astype(np.float32)
    assert full.shape == (B, M, M) and full.dtype == np.float32
    return full, res


def kernel(**inputs) -> np.ndarray:
    out, _ = run(inputs, trace=False)
    return out


# revision 14
# speedup vs baseline: 1.1389x; 1.0005x over previous
"""Self-contained Trainium2 Bass kernel for nn_CA_9363028705415 (sparse_attention).

Computes, per batch b:
    Q = relu(x[b] @ qW1 + qb1) @ qW2 + qb2          # [M, K]
    Kt = relu(x[b] @ kW1 + kb1) @ kW2 + kb2         # [M, K]
    S = Q @ Kt.T                                    # [M, M]
    out[b] = softmax(S / rowmax(S), axis=-1)        # max-DIVISION normalization

Shapes: B=16, M=2048, D=128, H=256, K=64.  Output [16, 2048, 2048] f32 (256 MB).

Sharding: data-parallel over batch across 8 NeuronCores; 2 batches/core; tiny
MLP weights replicated.  Single NEFF run SPMD via run_bass_kernel_spmd.

Device writes the output in fp16 (16 MB/core instead of 32 MB); the host
upcasts to f32 after gathering.  fp16 quantization error (~3e-4 rel) is far
below the 2e-2 gate.

x never touches the compute engines: a SWDGE cast-DMA produces a bf16 copy of
each x token-half in DRAM scratch, and a HWDGE xbar transpose-DMA loads
x^T [D, M] straight into SBUF (per-half tiles so the casts/transposes/mlp
pipeline at half granularity).

S is computed in two [128, 1024] PSUM half-tiles (2 banks each; psum_s pool
2 bufs = 4 banks) so the OTHER 4 banks serve a dedicated MLP pool: batch 1's
MLP chunks interleave into batch 0's S loop without stealing S-pipeline slots.

Per 128-row tile:
  PE:  2x2 matmuls -> two [128,1024] f32 PSUM halves
  DVE: per half, fused PSUM->SBUF fp16 copy + running row-max
       (tensor_scalar accum_out=max, 1x mode: fp32 PSUM source);
       reduce_max over the two half-maxes; reciprocal of max ONLY
  ACT: exp(S * (1/max)) over the full fp16 row, fused row-sum accumulate
  DVE: reciprocal of previous tile's sum (separate op so exp never waits on
       the previous accumulator read), then the previous tile's normalize
       multiply at 4x (fp16 in/out SBUF); some norms go to ACT (1x) to
       balance -- NORM_PATTERN
  HWDGE DMA: 1 MB fp16 output chunks (2 row-tiles; final tile split for tail)
"""

import numpy as np
import ml_dtypes

import concourse.bass as bass
import concourse.mybir as mybir
from concourse import bacc
import concourse.tile as tile
from concourse.bass import ts
from concourse.bass_utils import run_bass_kernel_spmd

F32 = mybir.dt.float32
BF16 = mybir.dt.bfloat16
FP16 = mybir.dt.float16
AF = mybir.ActivationFunctionType
ALU = mybir.AluOpType

N_CORES = 8
B, M, D, H, KF = 16, 2048, 128, 256, 64
BPC = B // N_CORES     # batches per core
MT = M // 128          # 16 row-tiles per batch
HM = M // 2            # 1024: half-tile free size

# normalize engine per row-tile: DVE fp16->fp16 runs 4x (~0.6us/tile),
# ACT copy-with-scale is 1x (~2us/tile); ~11/16 on DVE balances the
# engines given ACT also owns the exp.
NORM_PATTERN = (
    "dve", "act", "dve", "dve", "act", "dve", "dve", "act",
    "dve", "dve", "act", "dve", "dve", "act", "dve", "dve",
)


def _evac_bias(nc, engine, out, in_, bias, relu):
    """out = [relu](in_ + bias), bias is [P,1] per-partition AP."""
    if engine == "act":
        nc.scalar.activation(
            out, in_, AF.Relu if relu else AF.Identity, bias=bias, scale=1.0
        )
    else:
        if relu:
            nc.vector.tensor_scalar(out, in_, bias, 0.0, op0=ALU.add, op1=ALU.max)
        else:
            nc.vector.tensor_scalar(out, in_, bias, None, op0=ALU.add)


def _norm(nc, engine, out, t, isum):
    if engine == "act":
        nc.scalar.mul(out, t, isum)
    else:
        nc.vector.tensor_scalar_mul(out, t, isum)


def build_nc():
    nc = bacc.Bacc()

    x = nc.dram_tensor("x", [BPC, M, D], F32, kind="ExternalInput")
    w1d, b1d, w2d, b2d = {}, {}, {}, {}
    for h in ("q", "k"):
        w1d[h] = nc.dram_tensor(f"{h}W1", [D, H], F32, kind="ExternalInput")
        b1d[h] = nc.dram_tensor(f"{h}b1", [H], F32, kind="ExternalInput")
        w2d[h] = nc.dram_tensor(f"{h}W2", [H, KF], F32, kind="ExternalInput")
        b2d[h] = nc.dram_tensor(f"{h}b2", [KF], F32, kind="ExternalInput")
    out = nc.dram_tensor("out", [BPC, M, M], FP16, kind="ExternalOutput")

    ident_np = np.eye(128, dtype=ml_dtypes.bfloat16)
    ident_dram = nc.inline_tensor(ident_np, name="ident_data")

    # [p, n, d]: batch-0 token (n*128+p), feature d
    x0_r = x[0].rearrange("(n p) d -> p n d", p=128)
    # [b, p, n, m]: out[b, n*128+p, m]
    out_r = out[:].rearrange("b (n p) m -> b p n m", p=128)

    with tile.TileContext(nc) as tc:
        with (
            tc.tile_pool(name="consts", bufs=1) as consts,
            tc.tile_pool(name="xstage", bufs=1, space="DRAM") as xstage,
            tc.tile_pool(name="xin", bufs=1) as xin_pool,
            tc.tile_pool(name="xt", bufs=1) as xt_pool,
            tc.tile_pool(name="ht", bufs=2) as ht_pool,
            tc.tile_pool(name="qkt", bufs=2) as qkt_pool,
            tc.tile_pool(name="texp", bufs=4) as t_pool,
            tc.tile_pool(name="osb", bufs=6) as out_pool,
            tc.tile_pool(name="small", bufs=8) as small_pool,
            tc.tile_pool(name="psum_s", bufs=2, space="PSUM") as psum_s,
            tc.tile_pool(name="psum_mlp", bufs=2, space="PSUM") as psum_mlp,
        ):
            norm_i = 0

            # ---- batch-0 x: SWDGE cast-DMA f32->bf16 straight into SBUF
            # (queue head so the ramp path gets the bytes first), then
            # batch-1 casts into DRAM scratch for the transpose-DMA route ----
            xsb = xin_pool.tile([128, 2, 8, 128], BF16, tag="xsb", name="xsb")
            for half in range(2):
                nc.gpsimd.dma_start(
                    out=xsb[:, half], in_=x0_r[:, half * 8 : (half + 1) * 8, :]
                )
            xbf, xT1 = {}, {}
            for half in range(2):
                xbf[half] = xstage.tile([HM, D], BF16, tag=f"xbf{half}", name="xbf")
                xT1[half] = xt_pool.tile([128, HM], BF16, tag=f"xt1{half}", name="xT1")
                nc.gpsimd.dma_start(out=xbf[half], in_=x[1][ts(half, HM), :])

            # ---- weights/biases via HWDGE f32 loads + tiny engine casts ----
            ident = consts.tile([128, 128], BF16, tag="ident")
            nc.sync.dma_start(out=ident, in_=ident_dram[:])
            w1, w2, b1, b2 = {}, {}, {}, {}
            wraw = {}
            for h in ("q", "k"):
                wraw[h, 1] = consts.tile([D, H], F32, tag=f"w1r{h}", name=f"w1r{h}")
                nc.sync.dma_start(out=wraw[h, 1], in_=w1d[h][:])
                wraw[h, 2] = consts.tile(
                    [128, 2, KF], F32, tag=f"w2r{h}", name=f"w2r{h}"
                )
                nc.sync.dma_start(
                    out=wraw[h, 2], in_=w2d[h][:].rearrange("(c p) k -> p c k", p=128)
                )
                b1[h] = consts.tile([128, 2], F32, tag=f"b1{h}", name=f"b1{h}")
                nc.sync.dma_start(
                    out=b1[h], in_=b1d[h][:].rearrange("(c p) -> p c", p=128)
                )
                b2[h] = consts.tile([KF, 1], F32, tag=f"b2{h}", name=f"b2{h}")
                nc.sync.dma_start(
                    out=b2[h], in_=b2d[h][:].rearrange("(k o) -> k o", o=1)
                )
            # batch-1 transpose-DMAs (sync queue, after the small loads)
            for half in range(2):
                nc.sync.dma_start_transpose(out=xT1[half], in_=xbf[half])
            for h in ("q", "k"):
                w1[h] = consts.tile([D, H], BF16, tag=f"w1{h}", name=f"w1{h}")
                nc.vector.tensor_copy(w1[h], wraw[h, 1])
                w2[h] = consts.tile([128, 2, KF], BF16, tag=f"w2{h}", name=f"w2{h}")
                nc.vector.tensor_copy(w2[h], wraw[h, 2])

            # ---- PE warm-up: dummy matmuls trip the HAM clock gate to
            # 2.4 GHz before the real MLP starts ----
            wu = consts.tile([128, 512], BF16, tag="wu", name="warm")
            nc.vector.memset(wu, 0.0)
            for i in range(8):
                ps_w = psum_s.tile([128, 512], F32, tag="ps", name="ps_warm")
                nc.tensor.matmul(
                    ps_w, lhsT=wu[:, 0:128], rhs=wu, start=True, stop=True
                )

            # ---- batch-0 xT: DVE cast + warm-PE transposes, per half ----
            xT0 = xt_pool.tile([128, M], BF16, tag="xt0", name="xT0")
            xsb = xin_pool.tile([128, 2, 8, 128], BF16, tag="xsb", name="xsb")

            def b0_prep(half):
                nc.vector.tensor_copy(xsb[:, half], xf[half])
                tp = psum_mlp.tile([128, HM], BF16, tag="ps", name="tp")
                for it in range(8):
                    nc.tensor.transpose(
                        tp[:, ts(it, 128)], xsb[:, half, it, :], ident
                    )
                for e, fc in (("act", 0), ("dve", 1)):
                    dst = xT0[:, half * HM + fc * 512 : half * HM + (fc + 1) * 512]
                    if e == "act":
                        nc.scalar.copy(dst, tp[:, ts(fc, 512)])
                    else:
                        nc.vector.tensor_copy(dst, tp[:, ts(fc, 512)])

            def phase_a_chunks(b):
                """MLP chunk closures for batch b (fine-grained so they can
                interleave into the previous batch's S loop).  Each chunk uses
                one psum_mlp slot (2 banks)."""
                if b == 0:
                    xT_ap = lambda half, fc: xT0[:, half * HM + fc * 512 : half * HM + (fc + 1) * 512]
                else:
                    xT_ap = lambda half, fc: xT1[half][:, ts(fc, 512)]
                ht = {}
                for h in ("q", "k"):
                    ht[h] = ht_pool.tile(
                        [128, 2, M], BF16, tag=f"ht{h}", name=f"ht{h}"
                    )
                qkt = {}
                for h in ("q", "k"):
                    qkt[h] = qkt_pool.tile(
                        [KF, M], BF16, tag=f"qkt{h}", name=f"qkt{h}"
                    )

                def c_mlp1(h, pc, half):
                    def go():
                        ps1 = psum_mlp.tile([128, HM], F32, tag="ps", name="ps1")
                        for fc in range(2):
                            nc.tensor.matmul(
                                ps1[:, ts(fc, 512)],
                                lhsT=w1[h][:, ts(pc, 128)],
                                rhs=xT_ap(half, fc),
                                start=True,
                                stop=True,
                            )
                        for e, fc in (("act", 0), ("dve", 1)):
                            _evac_bias(
                                nc,
                                e,
                                ht[h][:, pc, ts(half * 2 + fc, 512)],
                                ps1[:, ts(fc, 512)],
                                b1[h][:, pc : pc + 1],
                                relu=True,
                            )
                    return go

                def c_mlp2(h, mh):
                    def go():
                        ps2 = psum_mlp.tile([KF, HM], F32, tag="ps", name="ps2")
                        for fc in range(2):
                            for kc in range(2):
                                nc.tensor.matmul(
                                    ps2[:, ts(fc, 512)],
                                    lhsT=w2[h][:, kc, :],
                                    rhs=ht[h][:, kc, ts(mh * 2 + fc, 512)],
                                    start=(kc == 0),
                                    stop=(kc == 1),
                                )
                        for e, fc in (("act", 0), ("dve", 1)):
                            _evac_bias(
                                nc,
                                e,
                                qkt[h][:, ts(mh * 2 + fc, 512)],
                                ps2[:, ts(fc, 512)],
                                b2[h],
                                relu=False,
                            )
                    return go

                chunks = []
                for half in range(2):
                    for h, pc in (("q", 0), ("k", 0), ("q", 1), ("k", 1)):
                        chunks.append(c_mlp1(h, pc, half))
                for mh in range(2):
                    for h in ("q", "k"):
                        chunks.append(c_mlp2(h, mh))
                return qkt, chunks

            def s_loop(b, qkt, next_chunks):
                """S + softmax loop for batch b, interleaving next_chunks
                (next batch's MLP) into the iterations."""
                nonlocal norm_i
                pending = None  # (rt, t_tile, sum_tile)

                def finish(j, t_j, isum_ap):
                    nonlocal norm_i
                    osb = out_pool.tile([128, M], FP16, tag="o", name="osb")
                    _norm(
                        nc,
                        NORM_PATTERN[norm_i % len(NORM_PATTERN)],
                        osb,
                        t_j,
                        isum_ap,
                    )
                    norm_i += 1
                    if j == MT - 1:
                        # tail: split the last tile's DMA in half
                        for hh in range(2):
                            nc.sync.dma_start(
                                out=out_r[b][:, j : j + 1, ts(hh, HM)],
                                in_=osb[:, ts(hh, HM)],
                            )
                    else:
                        nc.sync.dma_start(out=out_r[b][:, j : j + 1, :], in_=osb)

                for rt in range(MT):
                    sc_t = t_pool.tile([128, M], FP16, tag="sc", name="sc")
                    mx2 = small_pool.tile([128, 2], F32, tag="mx", name="mx2")
                    for hf in range(2):
                        ps_s = psum_s.tile([128, HM], F32, tag="ps", name="ps_s")
                        for fc in range(2):
                            nc.tensor.matmul(
                                ps_s[:, ts(fc, 512)],
                                lhsT=qkt["q"][:, ts(rt, 128)],
                                rhs=qkt["k"][:, ts(hf * 2 + fc, 512)],
                                start=True,
                                stop=True,
                            )
                        # fused PSUM->SBUF fp16 evac + row-max of this half
                        nc.vector.tensor_scalar(
                            sc_t[:, ts(hf, HM)],
                            ps_s,
                            0.0,
                            None,
                            op0=ALU.add,
                            op1=ALU.max,
                            accum_out=mx2[:, hf : hf + 1],
                        )
                    mx = small_pool.tile([128, 1], F32, tag="m1", name="mx")
                    nc.vector.reduce_max(mx, mx2, axis=mybir.AxisListType.X)
                    imax = small_pool.tile([128, 1], F32, tag="im", name="imax")
                    nc.vector.reciprocal(imax, mx)

                    sum_t = small_pool.tile([128, 1], F32, tag="sm", name="sum")
                    t_t = t_pool.tile([128, M], FP16, tag="t")
                    nc.scalar.activation(
                        t_t,
                        sc_t,
                        AF.Exp,
                        bias=0.0,
                        scale=imax,
                        accum_out=sum_t,
                    )

                    if pending is not None:
                        pj, pt, psum_t = pending
                        isum = small_pool.tile([128, 1], F32, tag="is", name="isum")
                        nc.vector.reciprocal(isum, psum_t)
                        finish(pj, pt, isum)
                    pending = (rt, t_t, sum_t)

                    if next_chunks:
                        next_chunks.pop(0)()

                pj, pt, psum_t = pending
                isum = small_pool.tile([128, 1], F32, tag="is", name="isum")
                nc.vector.reciprocal(isum, psum_t)
                finish(pj, pt, isum)
                while next_chunks:
                    next_chunks.pop(0)()

            qkt0, chunks0 = phase_a_chunks(0)
            # A0: prep half 0, its mlp1 chunks, prep half 1, the rest
            b0_prep(0)
            for c in chunks0[0:4]:
                c()
            b0_prep(1)
            for c in chunks0[4:]:
                c()
            qkt1, chunks1 = phase_a_chunks(1)
            s_loop(0, qkt0, chunks1)
            s_loop(1, qkt1, [])
    nc.finalize()
    return nc


_NC_CACHE = None


def _get_nc():
    global _NC_CACHE
    if _NC_CACHE is None:
        _NC_CACHE = build_nc()
    return _NC_CACHE


def run(inputs, trace=False, trace_cores=None):
    """Run on 8 cores; returns (full_output [B,M,M] f32, BassKernelResults)."""
    nc = _get_nc()
    in_maps = []
    x = np.ascontiguousarray(inputs["x"], dtype=np.float32)
    for c in range(N_CORES):
        im = {"x": np.ascontiguousarray(x[c * BPC : (c + 1) * BPC])}
        for k in ("qW1", "qb1", "qW2", "qb2", "kW1", "kb1", "kW2", "kb2"):
            im[k] = np.ascontiguousarray(inputs[k], dtype=np.float32)
        in_maps.append(im)
    res = run_bass_kernel_spmd(
        nc,
        in_maps,
        core_ids=list(range(N_CORES)),
        trace=trace,
        trace_cores=trace_cores,
    )
    outs = [np.asarray(r["out"]) for r in res.results]
    full = np.concatenate(outs, axis=0).astype(np.float32)
    assert full.shape == (B, M, M) and full.dtype == np.float32
    return full, res


def kernel(**inputs) -> np.ndarray:
    out, _ = run(inputs, trace=False)
    return out
